# revision 14
# baseline (speedup 1.0000x reference)
"""NetVLAD Trainium2 Bass kernel.

Full-input contract: kernel(**inputs) takes the complete fp32 tensors
  x         [64, 128, 64, 64]
  conv_w    [32, 128]
  centroids [32, 128]
and returns the full [64, 4096] fp32 output.

Sharding: data-parallel over the batch N=64 across 8 cores (8 images per
core); the small parameters are replicated (pre-transposed/tiled on host).

Per-core dataflow (per image, P = H*W = 4096 pixels, D = 128, K = 32):
  1. SWDGE DMA loads x[n] as [128(D), 4096(P)] casting fp32->bf16.
  2. PE: logitsT chunks: out[p,k] via lhsT=x_chunk, rhs=conv_w^T  (PSUM fp32)
  3. PE: transpose x chunks -> xT [p, d] (bf16, PSUM), ACT copies to SBUF
  4. GPSIMD/DVE: fused square+row-reduce of xT chunks -> normsq [128, 32]
  5. ACT: rn = exp(-0.5*log(normsq))   (1/||x_p||, log/exp only - one table set)
  6. DVE: lTs = logitsT * rn (bcast), ACT: eT = exp(lTs),
     DVE: s = rowsums, s_inv, t = rn*s_inv, aT = eT*s_inv, bT = eT*t
  7. PE: aggT[d,k] += xT_c^T @ bT_c ; asum[*,k] += ones^T @ aT_c (bcast rows)
  8. batched finale over all 8 images: vlad = agg - asum*c, intra-L2,
     global L2, transpose to [k, d] and DMA out.
"""

import os
import sys
import numpy as np

if "/opt/trn_rl_repo" not in sys.path:
    sys.path.insert(0, "/opt/trn_rl_repo")

import ml_dtypes
import concourse.bass as bass
import concourse.bacc as bacc
import concourse.mybir as mybir
import concourse.tile as tile
from concourse.bass_utils import run_bass_kernel_spmd

dt = mybir.dt
ALU = mybir.AluOpType
ACTF = mybir.ActivationFunctionType

N = 64
D = 128
HW = 4096          # H*W
K = 32
N_CORES = 8
IMGS = N // N_CORES  # 8 images per core
NCH = HW // 128      # 32 chunks of 128 pixels

# norm-square chunk split: GP squares SQ_GP chunks, DVE squares SQ_DVE,
# ACT does SQ_ACT chunks fully (Square + accum); DVE reduces the squared ones
SQ_GP = 13
SQ_DVE = 13
SQ_ACT = NCH - SQ_GP - SQ_DVE


def _emit(tc: "tile.TileContext", io: dict):
    nc = tc.nc
    ctxs = []

    x_d = io["x"]          # [IMGS, 128, 4096] f32 DRAM
    wT_d = io["wT"]        # [128, 32] bf16
    identb_d = io["identb"]  # [128, 128] bf16
    onesb_d = io["onesb"]    # [128, 128] bf16
    identf_d = io["identf"]  # [128, 128] f32
    onesf_d = io["onesf"]    # [128, 128] f32
    cT8_d = io["cT8"]        # [128, 256] f32
    out_d = io["out"]        # [IMGS, 4096] f32

    from contextlib import ExitStack
    es = ExitStack()

    const = es.enter_context(tc.tile_pool(name="const", bufs=1))
    wT = const.tile([128, K], dt.bfloat16, tag="wT")
    identb = const.tile([128, 128], dt.bfloat16, tag="identb")
    onesb = const.tile([128, 128], dt.bfloat16, tag="onesb")
    identf = const.tile([128, 128], dt.float32, tag="identf")
    onesf = const.tile([128, 128], dt.float32, tag="onesf")
    cT8 = const.tile([128, IMGS * K], dt.float32, tag="cT8")
    nc.sync.dma_start(out=wT[:], in_=wT_d)
    nc.sync.dma_start(out=identb[:], in_=identb_d)
    nc.sync.dma_start(out=onesb[:], in_=onesb_d)
    nc.sync.dma_start(out=identf[:], in_=identf_d)
    nc.sync.dma_start(out=onesf[:], in_=onesf_d)
    nc.sync.dma_start(out=cT8[:], in_=cT8_d)

    # SBUF pools (image pipeline)
    xb_pool = es.enter_context(tc.tile_pool(name="xb", bufs=2))
    xts_pool = es.enter_context(tc.tile_pool(name="xts", bufs=2))
    soft_pool = es.enter_context(tc.tile_pool(name="soft", bufs=2))
    small_pool = es.enter_context(tc.tile_pool(name="small", bufs=2))
    scr_pool = es.enter_context(tc.tile_pool(name="scr", bufs=8))
    acc_pool = es.enter_context(tc.tile_pool(name="acc", bufs=1))

    agg8 = acc_pool.tile([128, IMGS * K], dt.float32, tag="agg8")
    asum8 = acc_pool.tile([128, IMGS * K], dt.float32, tag="asum8")

    # PSUM pools for the per-image phase (8 banks total available)
    with tc.tile_pool(name="lgp", bufs=3, space="PSUM") as lg_pool, \
         tc.tile_pool(name="xtp", bufs=2, space="PSUM") as xt_pool, \
         tc.tile_pool(name="aggp", bufs=2, space="PSUM") as agg_pool, \
         tc.tile_pool(name="asump", bufs=1, space="PSUM") as asum_pool:

        for i in range(IMGS):
            # 1. load x[i] (bf16, host-cast) [128, 4096]
            xb = xb_pool.tile([128, HW], dt.bfloat16, tag="xb")
            nc.sync.dma_start(out=xb[:], in_=x_d[i])

            # 2. logitsT: two PSUM banks of 16 chunks each
            lgs = []
            for h in range(2):
                lg = lg_pool.tile([128, 512], dt.float32, tag="lg")
                lgs.append(lg)
                for j in range(16):
                    c = 16 * h + j
                    nc.tensor.matmul(
                        lg[:, 32 * j:32 * j + 32],
                        lhsT=xb[:, 128 * c:128 * c + 128],
                        rhs=wT[:],
                        start=(j == 0),
                        stop=(j == 15),
                    )

            # 3. transpose x chunks (bf16) and copy to SBUF
            xts = xts_pool.tile([128, HW], dt.bfloat16, tag="xts")
            for b in range(4):
                xtp = xt_pool.tile([128, 1024], dt.bfloat16, tag="xt")
                for j in range(8):
                    c = 8 * b + j
                    nc.tensor.matmul(
                        xtp[:, 128 * j:128 * j + 128],
                        lhsT=xb[:, 128 * c:128 * c + 128],
                        rhs=identb[:],
                        is_transpose=True,
                        start=(j == 0),
                        stop=(j == 7),
                    )
                nc.scalar.copy(out=xts[:, 1024 * b:1024 * b + 1024], in_=xtp[:])

            # 4. norm-squares: square on GP/DVE + grouped reduce on DVE;
            #    remaining chunks fully on ACT (Square with accum_out)
            norms = small_pool.tile([128, NCH], dt.float32, tag="norms")
            sqa = scr_pool.tile([128, SQ_GP * 128], dt.bfloat16, tag="sqa")
            nc.gpsimd.tensor_tensor(
                out=sqa[:], in0=xts[:, :SQ_GP * 128],
                in1=xts[:, :SQ_GP * 128], op=ALU.mult)
            sqb = scr_pool.tile([128, SQ_DVE * 128], dt.bfloat16, tag="sqb")
            o0 = SQ_GP * 128
            nc.vector.tensor_tensor(
                out=sqb[:], in0=xts[:, o0:o0 + SQ_DVE * 128],
                in1=xts[:, o0:o0 + SQ_DVE * 128], op=ALU.mult)
            nc.vector.tensor_reduce(
                out=norms[:, 0:SQ_GP],
                in_=sqa[:].rearrange("p (c d) -> p c d", d=128),
                axis=mybir.AxisListType.X, op=ALU.add)
            nc.vector.tensor_reduce(
                out=norms[:, SQ_GP:SQ_GP + SQ_DVE],
                in_=sqb[:].rearrange("p (c d) -> p c d", d=128),
                axis=mybir.AxisListType.X, op=ALU.add)
            for j in range(SQ_ACT):
                c = SQ_GP + SQ_DVE + j
                scr = scr_pool.tile([128, 128], dt.bfloat16, tag="scr")
                nc.scalar.activation(
                    scr[:], xts[:, 128 * c:128 * c + 128], ACTF.Square,
                    accum_out=norms[:, c:c + 1],
                )

            # 5. rn = normsq^-0.5 via exp(-0.5*log(normsq)) (log/exp table set)
            lnn = small_pool.tile([128, NCH], dt.float32, tag="lnn")
            nc.scalar.activation(lnn[:], norms[:], ACTF.Ln)
            rn = small_pool.tile([128, NCH], dt.float32, tag="rn")
            nc.scalar.activation(rn[:], lnn[:], ACTF.Exp, scale=-0.5)

            # 6. softmax over k (free dim), pixel-major
            lTs = soft_pool.tile([128, NCH * K], dt.bfloat16, tag="lTs")
            for h in range(2):
                nc.vector.tensor_tensor(
                    out=lTs[:, 512 * h:512 * h + 512].rearrange(
                        "p (c k) -> p c k", k=K),
                    in0=lgs[h][:].rearrange("p (c k) -> p c k", k=K),
                    in1=rn[:, 16 * h:16 * h + 16].unsqueeze(2).broadcast_to(
                        (128, 16, K)),
                    op=ALU.mult,
                )
            eT = soft_pool.tile([128, NCH * K], dt.bfloat16, tag="eT")
            nc.scalar.activation(eT[:], lTs[:], ACTF.Exp)

            s = small_pool.tile([128, NCH], dt.float32, tag="s")
            nc.vector.tensor_reduce(
                out=s[:], in_=eT[:].rearrange("p (c k) -> p c k", k=K),
                axis=mybir.AxisListType.X, op=ALU.add,
            )
            s_inv = small_pool.tile([128, NCH], dt.float32, tag="s_inv")
            nc.vector.reciprocal(s_inv[:], s[:])
            t = small_pool.tile([128, NCH], dt.float32, tag="t")
            nc.vector.tensor_tensor(out=t[:], in0=rn[:], in1=s_inv[:],
                                    op=ALU.mult)

            aT = soft_pool.tile([128, NCH * K], dt.bfloat16, tag="aT")
            nc.gpsimd.tensor_tensor(
                out=aT[:].rearrange("p (c k) -> p c k", k=K),
                in0=eT[:].rearrange("p (c k) -> p c k", k=K),
                in1=s_inv[:].unsqueeze(2).broadcast_to((128, NCH, K)),
                op=ALU.mult,
            )
            bT = soft_pool.tile([128, NCH * K], dt.bfloat16, tag="bT")
            nc.gpsimd.tensor_tensor(
                out=bT[:].rearrange("p (c k) -> p c k", k=K),
                in0=eT[:].rearrange("p (c k) -> p c k", k=K),
                in1=t[:].unsqueeze(2).broadcast_to((128, NCH, K)),
                op=ALU.mult,
            )

            # 7. aggT[d,k] and asum (broadcast over partitions)
            aggp = agg_pool.tile([128, K], dt.float32, tag="agg")
            for c in range(NCH):
                nc.tensor.matmul(
                    aggp[:],
                    lhsT=xts[:, 128 * c:128 * c + 128],
                    rhs=bT[:, K * c:K * c + K],
                    start=(c == 0), stop=(c == NCH - 1),
                )
            asump = asum_pool.tile([128, K], dt.float32, tag="asum")
            for c in range(NCH):
                nc.tensor.matmul(
                    asump[:],
                    lhsT=onesb[:],
                    rhs=aT[:, K * c:K * c + K],
                    start=(c == 0), stop=(c == NCH - 1),
                )
            nc.scalar.copy(out=agg8[:, K * i:K * i + K], in_=aggp[:])
            nc.vector.tensor_copy(asum8[:, K * i:K * i + K], asump[:])

    # 8. batched finale over all images
    fin_pool = es.enter_context(tc.tile_pool(name="fin", bufs=1))
    KF = IMGS * K  # 256

    v1 = fin_pool.tile([128, KF], dt.float32, tag="v1")
    nc.vector.tensor_tensor(out=v1[:], in0=asum8[:], in1=cT8[:], op=ALU.mult)
    vlad = fin_pool.tile([128, KF], dt.float32, tag="vlad")
    nc.vector.tensor_tensor(out=vlad[:], in0=agg8[:], in1=v1[:],
                            op=ALU.subtract)
    sqv = fin_pool.tile([128, KF], dt.float32, tag="sqv")
    nc.vector.tensor_tensor(out=sqv[:], in0=vlad[:], in1=vlad[:], op=ALU.mult)

    with tc.tile_pool(name="finp", bufs=2, space="PSUM") as finp_pool, \
         tc.tile_pool(name="otp", bufs=2, space="PSUM") as ot_pool:
        nrm2p = finp_pool.tile([128, KF], dt.float32, tag="red")
        nc.tensor.matmul(nrm2p[:], lhsT=onesf[:], rhs=sqv[:],
                         start=True, stop=True)
        lnr = fin_pool.tile([128, KF], dt.float32, tag="lnr")
        nc.scalar.activation(lnr[:], nrm2p[:], ACTF.Ln)
        rinv = fin_pool.tile([128, KF], dt.float32, tag="rinv")
        nc.scalar.activation(rinv[:], lnr[:], ACTF.Exp, scale=-0.5)

        vn = fin_pool.tile([128, KF], dt.float32, tag="vn")
        nc.vector.tensor_tensor(out=vn[:], in0=vlad[:], in1=rinv[:],
                                op=ALU.mult)
        sqn = fin_pool.tile([128, KF], dt.float32, tag="sqn")
        nc.vector.tensor_tensor(out=sqn[:], in0=vn[:], in1=vn[:], op=ALU.mult)

        gp = finp_pool.tile([128, KF], dt.float32, tag="red")
        nc.tensor.matmul(gp[:], lhsT=onesf[:], rhs=sqn[:],
                         start=True, stop=True)
        g2 = fin_pool.tile([128, IMGS], dt.float32, tag="g2")
        nc.vector.tensor_reduce(
            out=g2[:], in_=gp[:].rearrange("p (n k) -> p n k", k=K),
            axis=mybir.AxisListType.X, op=ALU.add,
        )
        lg2 = fin_pool.tile([128, IMGS], dt.float32, tag="lg2")
        nc.scalar.activation(lg2[:], g2[:], ACTF.Ln)
        ginv = fin_pool.tile([128, IMGS], dt.float32, tag="ginv")
        nc.scalar.activation(ginv[:], lg2[:], ACTF.Exp, scale=-0.5)

        fin = fin_pool.tile([128, KF], dt.float32, tag="fin")
        nc.vector.tensor_tensor(
            out=fin[:].rearrange("p (n k) -> p n k", k=K),
            in0=vn[:].rearrange("p (n k) -> p n k", k=K),
            in1=ginv[:].unsqueeze(2).broadcast_to((128, IMGS, K)),
            op=ALU.mult,
        )

        # transpose [128d, 32k] -> [32k, 128d] per image, pack 4 per bank
        for b in range(2):
            otp = ot_pool.tile([32, 512], dt.float32, tag="ot")
            for j in range(4):
                i = 4 * b + j
                nc.tensor.matmul(
                    otp[:, 128 * j:128 * j + 128],
                    lhsT=fin[:, K * i:K * i + K],
                    rhs=identf[:],
                    is_transpose=True,
                    start=(j == 0), stop=(j == 3),
                )
            osb = fin_pool.tile([32, 512], dt.float32, tag="osb")
            nc.scalar.copy(out=osb[:], in_=otp[:])
            nc.sync.dma_start(
                out=out_d[4 * b:4 * b + 4].rearrange("n (k d) -> k n d", k=K),
                in_=osb[:].rearrange("k (n d) -> k n d", n=4),
            )

    es.close()


def build_nc(debug: bool = False):
    nc = bacc.Bacc(
        "TRN2",
        target_bir_lowering=False,
        debug=debug,
        enable_asserts=debug,
        num_devices=N_CORES,
    )
    io = {
        "x": nc.dram_tensor("x", [IMGS, D, HW], dt.bfloat16,
                            kind="ExternalInput").ap(),
        "wT": nc.dram_tensor("wT", [128, K], dt.bfloat16,
                             kind="ExternalInput").ap(),
        "identb": nc.dram_tensor("identb", [128, 128], dt.bfloat16,
                                 kind="ExternalInput").ap(),
        "onesb": nc.dram_tensor("onesb", [128, 128], dt.bfloat16,
                                kind="ExternalInput").ap(),
        "identf": nc.dram_tensor("identf", [128, 128], dt.float32,
                                 kind="ExternalInput").ap(),
        "onesf": nc.dram_tensor("onesf", [128, 128], dt.float32,
                                kind="ExternalInput").ap(),
        "cT8": nc.dram_tensor("cT8", [128, IMGS * K], dt.float32,
                              kind="ExternalInput").ap(),
        "out": nc.dram_tensor("out", [IMGS, HW], dt.float32,
                              kind="ExternalOutput").ap(),
    }
    with tile.TileContext(nc) as tc:
        _emit(tc, io)
    nc.finalize()
    return nc


def make_in_maps(x, conv_w, centroids):
    """Host-side shard + param prep. x [64,128,64,64] f32."""
    bf = ml_dtypes.bfloat16
    xf = np.ascontiguousarray(x.reshape(N, D, HW)).astype(np.float32).astype(bf)
    consts = {
        "wT": np.ascontiguousarray(conv_w.T).astype(bf),
        "identb": np.eye(128, dtype=np.float32).astype(bf),
        "onesb": np.ones((128, 128), np.float32).astype(bf),
        "identf": np.eye(128, dtype=np.float32),
        "onesf": np.ones((128, 128), np.float32),
        "cT8": np.ascontiguousarray(
            np.tile(centroids.T.astype(np.float32), (1, IMGS))),
    }
    in_maps = []
    for core in range(N_CORES):
        m = dict(consts)
        m["x"] = np.ascontiguousarray(xf[IMGS * core:IMGS * (core + 1)])
        in_maps.append(m)
    return in_maps


_NC_CACHE = None


def kernel(x, conv_w, centroids):
    global _NC_CACHE
    if _NC_CACHE is None:
        _NC_CACHE = build_nc()
    nc = _NC_CACHE
    in_maps = make_in_maps(np.asarray(x), np.asarray(conv_w),
                           np.asarray(centroids))
    res = run_bass_kernel_spmd(nc, in_maps, list(range(N_CORES)))
    out = np.concatenate([np.asarray(res.results[i]["out"])
                          for i in range(N_CORES)], axis=0)
    return out.reshape(N, K * D).astype(np.float32)


# revision 23
# speedup vs baseline: 1.0291x; 1.0291x over previous
"""NetVLAD Trainium2 Bass kernel.

Full-input contract: kernel(**inputs) takes the complete fp32 tensors
  x         [64, 128, 64, 64]
  conv_w    [32, 128]
  centroids [32, 128]
and returns the full [64, 4096] fp32 output.

Sharding: data-parallel over the batch N=64 across 8 cores (8 images per
core); the small parameters are replicated (pre-transposed/tiled on host).

Per-core dataflow (per image, P = H*W = 4096 pixels, D = 128, K = 32):
  1. SWDGE DMA loads x[n] as [128(D), 4096(P)] casting fp32->bf16.
  2. PE: logitsT chunks: out[p,k] via lhsT=x_chunk, rhs=conv_w^T  (PSUM fp32)
  3. PE: transpose x chunks -> xT [p, d] (bf16, PSUM), ACT copies to SBUF
  4. GPSIMD/DVE: fused square+row-reduce of xT chunks -> normsq [128, 32]
  5. ACT: rn = exp(-0.5*log(normsq))   (1/||x_p||, log/exp only - one table set)
  6. DVE: lTs = logitsT * rn (bcast), ACT: eT = exp(lTs),
     DVE: s = rowsums, s_inv, t = rn*s_inv, aT = eT*s_inv, bT = eT*t
  7. PE: aggT[d,k] += xT_c^T @ bT_c ; asum[*,k] += ones^T @ aT_c (bcast rows)
  8. batched finale over all 8 images: vlad = agg - asum*c, intra-L2,
     global L2, transpose to [k, d] and DMA out.
"""

import os
import sys
import numpy as np

if "/opt/trn_rl_repo" not in sys.path:
    sys.path.insert(0, "/opt/trn_rl_repo")

import ml_dtypes
import concourse.bass as bass
import concourse.bacc as bacc
import concourse.mybir as mybir
import concourse.tile as tile
from concourse.bass_utils import run_bass_kernel_spmd

dt = mybir.dt
ALU = mybir.AluOpType
ACTF = mybir.ActivationFunctionType

N = 64
D = 128
HW = 4096          # H*W
K = 32
N_CORES = 8
IMGS = N // N_CORES  # 8 images per core
NCH = HW // 128      # 32 chunks of 128 pixels

# norm-square chunk split: GP squares SQ_GP chunks, DVE squares SQ_DVE,
# ACT does SQ_ACT chunks fully (Square + accum); DVE reduces the squared ones
SQ_GP = 16
SQ_DVE = 16
SQ_ACT = NCH - SQ_GP - SQ_DVE


def _patch_act_tables():
    """Force every activation we use (Exp/Ln/Square/Copy) onto the single
    'natural_log_exp_and_others' table set so the kernel loads ACT tables
    once instead of ping-ponging between sets (measured 27us of
    ACT_TABLE_LOAD otherwise). Set ids stay aligned with act_info.json:
    only the *contents* used for set-selection are filtered."""
    import concourse.hw_specs as hw_specs
    if getattr(hw_specs, "_netvlad_act_patch", False):
        return
    orig = hw_specs.get_activation_tables
    mine = {ACTF.Exp, ACTF.Ln, ACTF.Square, ACTF.Copy, ACTF.Identity}

    def patched(arch):
        tabs = orig(arch)
        out = {}
        for name, funcs in tabs.items():
            if name == "natural_log_exp_and_others":
                out[name] = funcs
            else:
                out[name] = funcs - mine
        return out

    hw_specs.get_activation_tables = patched
    bacc.get_activation_tables = patched
    hw_specs._netvlad_act_patch = True


def _emit(tc: "tile.TileContext", io: dict):
    nc = tc.nc
    ctxs = []

    x_d = io["x"]          # [IMGS, 128, 4096] bf16 DRAM
    wT_d = io["wT"]        # [128, 32] bf16
    identf_d = io["identf"]  # [128, 128] f32
    onesf_d = io["onesf"]    # [128, 128] f32
    cT8_d = io["cT8"]        # [128, 256] f32
    eb_d = io["eb"]          # [8, 1024] f32: kron(eye(8), ones(128))
    out_d = io["out"]        # [IMGS, 4096] f32

    from contextlib import ExitStack
    es = ExitStack()

    const = es.enter_context(tc.tile_pool(name="const", bufs=1))
    wT = const.tile([128, K], dt.bfloat16, tag="wT")
    identf = const.tile([128, 128], dt.float32, tag="identf")
    onesf = const.tile([128, 128], dt.float32, tag="onesf")
    cT8 = const.tile([128, IMGS * K], dt.float32, tag="cT8")
    eb = const.tile([8, IMGS * 128], dt.float32, tag="eb")
    nc.sync.dma_start(out=wT[:], in_=wT_d)
    nc.sync.dma_start(out=identf[:], in_=identf_d)
    nc.sync.dma_start(out=onesf[:], in_=onesf_d)
    nc.sync.dma_start(out=cT8[:], in_=cT8_d)
    nc.sync.dma_start(out=eb[:], in_=eb_d)

    # SBUF pools (image pipeline)
    xb_pool = es.enter_context(tc.tile_pool(name="xb", bufs=2))
    xts_pool = es.enter_context(tc.tile_pool(name="xts", bufs=2))
    soft_pool = es.enter_context(tc.tile_pool(name="soft", bufs=2))
    small_pool = es.enter_context(tc.tile_pool(name="small", bufs=2))
    scr_pool = es.enter_context(tc.tile_pool(name="scr", bufs=8))
    acc_pool = es.enter_context(tc.tile_pool(name="acc", bufs=1))

    agg8 = acc_pool.tile([128, IMGS * K], dt.float32, tag="agg8")
    asum8 = acc_pool.tile([K, IMGS], dt.float32, tag="asum8")

    # PSUM pools for the per-image phase (8 banks total available)
    with tc.tile_pool(name="lgp", bufs=3, space="PSUM") as lg_pool, \
         tc.tile_pool(name="aggp", bufs=2, space="PSUM") as agg_pool, \
         tc.tile_pool(name="asump", bufs=1, space="PSUM") as asum_pool:
        asum8_ps = asum_pool.tile([K, IMGS], dt.float32, tag="asum")

        for i in range(IMGS):
            # 1. load x[i] (bf16, host-cast) [128, 4096]
            xb = xb_pool.tile([128, HW], dt.bfloat16, tag="xb")
            nc.sync.dma_start(out=xb[:], in_=x_d[i])

            # 2. logitsT: two PSUM banks of 16 chunks each
            lgs = []
            for h in range(2):
                lg = lg_pool.tile([128, 512], dt.float32, tag="lg")
                lgs.append(lg)
                for j in range(16):
                    c = 16 * h + j
                    nc.tensor.matmul(
                        lg[:, 32 * j:32 * j + 32],
                        lhsT=xb[:, 128 * c:128 * c + 128],
                        rhs=wT[:],
                        start=(j == 0),
                        stop=(j == 15),
                    )

            # 3. transpose x -> xT chunk layout via the DMA xbar
            #    xts[p, (c, d)] = x[d, 128c + p]
            xts = xts_pool.tile([128, HW], dt.bfloat16, tag="xts")
            nc.sync.dma_start_transpose(
                out=xts[:].rearrange("p (c d) -> p c d", d=128),
                in_=xb[:],
            )

            # 4. norm-squares: square on GP/DVE + grouped reduce on DVE;
            #    remaining chunks fully on ACT (Square with accum_out)
            norms = small_pool.tile([128, NCH], dt.float32, tag="norms")
            sqa = scr_pool.tile([128, SQ_GP * 128], dt.bfloat16, tag="sqa")
            nc.gpsimd.tensor_tensor(
                out=sqa[:], in0=xts[:, :SQ_GP * 128],
                in1=xts[:, :SQ_GP * 128], op=ALU.mult)
            sqb = scr_pool.tile([128, SQ_DVE * 128], dt.bfloat16, tag="sqb")
            o0 = SQ_GP * 128
            nc.vector.tensor_tensor(
                out=sqb[:], in0=xts[:, o0:o0 + SQ_DVE * 128],
                in1=xts[:, o0:o0 + SQ_DVE * 128], op=ALU.mult)
            nc.vector.tensor_reduce(
                out=norms[:, 0:SQ_GP],
                in_=sqa[:].rearrange("p (c d) -> p c d", d=128),
                axis=mybir.AxisListType.X, op=ALU.add)
            nc.vector.tensor_reduce(
                out=norms[:, SQ_GP:SQ_GP + SQ_DVE],
                in_=sqb[:].rearrange("p (c d) -> p c d", d=128),
                axis=mybir.AxisListType.X, op=ALU.add)
            for j in range(SQ_ACT):
                c = SQ_GP + SQ_DVE + j
                scr = scr_pool.tile([128, 128], dt.bfloat16, tag="scr")
                nc.scalar.activation(
                    scr[:], xts[:, 128 * c:128 * c + 128], ACTF.Square,
                    accum_out=norms[:, c:c + 1],
                )

            # 5. rn = normsq^-0.5 via exp(-0.5*log(normsq)) (log/exp table set)
            lnn = small_pool.tile([128, NCH], dt.float32, tag="lnn")
            nc.scalar.activation(lnn[:], norms[:], ACTF.Ln)
            rn = small_pool.tile([128, NCH], dt.float32, tag="rn")
            nc.scalar.activation(rn[:], lnn[:], ACTF.Exp, scale=-0.5)

            # 6. softmax over k (free dim), pixel-major
            lTs = soft_pool.tile([128, NCH * K], dt.bfloat16, tag="lTs")
            for h in range(2):
                nc.vector.tensor_tensor(
                    out=lTs[:, 512 * h:512 * h + 512].rearrange(
                        "p (c k) -> p c k", k=K),
                    in0=lgs[h][:].rearrange("p (c k) -> p c k", k=K),
                    in1=rn[:, 16 * h:16 * h + 16].unsqueeze(2).broadcast_to(
                        (128, 16, K)),
                    op=ALU.mult,
                )
            eT = soft_pool.tile([128, NCH * K], dt.bfloat16, tag="eT")
            nc.scalar.activation(eT[:], lTs[:], ACTF.Exp)

            s = small_pool.tile([128, NCH], dt.float32, tag="s")
            nc.vector.tensor_reduce(
                out=s[:], in_=eT[:].rearrange("p (c k) -> p c k", k=K),
                axis=mybir.AxisListType.X, op=ALU.add,
            )
            s_inv = small_pool.tile([128, NCH], dt.float32, tag="s_inv")
            nc.vector.reciprocal(s_inv[:], s[:])
            s_inv_b = small_pool.tile([128, NCH], dt.bfloat16, tag="s_inv_b")
            nc.vector.tensor_copy(s_inv_b[:], s_inv[:])
            t = small_pool.tile([128, NCH], dt.float32, tag="t")
            nc.vector.tensor_tensor(out=t[:], in0=rn[:], in1=s_inv[:],
                                    op=ALU.mult)

            bT = soft_pool.tile([128, NCH * K], dt.bfloat16, tag="bT")
            nc.gpsimd.tensor_tensor(
                out=bT[:].rearrange("p (c k) -> p c k", k=K),
                in0=eT[:].rearrange("p (c k) -> p c k", k=K),
                in1=t[:].unsqueeze(2).broadcast_to((128, NCH, K)),
                op=ALU.mult,
            )

            # 7. aggT[d,k] += xT_c^T @ bT_c ; asum[k] += eT_c^T @ s_inv_c
            aggp = agg_pool.tile([128, K], dt.float32, tag="agg")
            for c in range(NCH):
                nc.tensor.matmul(
                    aggp[:],
                    lhsT=xts[:, 128 * c:128 * c + 128],
                    rhs=bT[:, K * c:K * c + K],
                    start=(c == 0), stop=(c == NCH - 1),
                )
            for c in range(NCH):
                nc.tensor.matmul(
                    asum8_ps[:, i:i + 1],
                    lhsT=eT[:, K * c:K * c + K],
                    rhs=s_inv_b[:, c:c + 1],
                    start=(c == 0), stop=(c == NCH - 1),
                )
            nc.scalar.copy(out=agg8[:, K * i:K * i + K], in_=aggp[:])
            nc.vector.tensor_copy(asum8[:, i:i + 1], asum8_ps[:, i:i + 1])

    # 8. batched finale over all images
    fin_pool = es.enter_context(tc.tile_pool(name="fin", bufs=1))
    KF = IMGS * K  # 256

    with tc.tile_pool(name="finp", bufs=2, space="PSUM") as finp_pool, \
         tc.tile_pool(name="otp", bufs=2, space="PSUM") as ot_pool:
        # broadcast asum over partitions: asum8 [32k, 8n] -> vbc [128d, (n k)]
        ast_ps = ot_pool.tile([8, K], dt.float32, tag="ast")
        nc.tensor.matmul(ast_ps[:], lhsT=asum8[:], rhs=identf[0:K, 0:K],
                         is_transpose=True, start=True, stop=True)
        ast = fin_pool.tile([8, K], dt.float32, tag="astsb")
        nc.scalar.copy(out=ast[:], in_=ast_ps[:])
        vbc_ps = finp_pool.tile([128, KF], dt.float32, tag="red")
        for n in range(IMGS):
            nc.tensor.matmul(
                vbc_ps[:, K * n:K * n + K],
                lhsT=eb[:, 128 * n:128 * n + 128],
                rhs=ast[:],
                start=(n == 0), stop=(n == IMGS - 1),
            )
        v1 = fin_pool.tile([128, KF], dt.float32, tag="v1")
        nc.vector.tensor_tensor(out=v1[:], in0=vbc_ps[:], in1=cT8[:],
                                op=ALU.mult)
        vlad = fin_pool.tile([128, KF], dt.float32, tag="vlad")
        nc.vector.tensor_tensor(out=vlad[:], in0=agg8[:], in1=v1[:],
                                op=ALU.subtract)
        sqv = fin_pool.tile([128, KF], dt.float32, tag="sqv")
        nc.vector.tensor_tensor(out=sqv[:], in0=vlad[:], in1=vlad[:],
                                op=ALU.mult)
        nrm2p = finp_pool.tile([128, KF], dt.float32, tag="red")
        nc.tensor.matmul(nrm2p[:], lhsT=onesf[:], rhs=sqv[:],
                         start=True, stop=True)
        lnr = fin_pool.tile([128, KF], dt.float32, tag="lnr")
        nc.scalar.activation(lnr[:], nrm2p[:], ACTF.Ln)
        rinv = fin_pool.tile([128, KF], dt.float32, tag="rinv")
        nc.scalar.activation(rinv[:], lnr[:], ACTF.Exp, scale=-0.5)

        vn = fin_pool.tile([128, KF], dt.float32, tag="vn")
        nc.vector.tensor_tensor(out=vn[:], in0=vlad[:], in1=rinv[:],
                                op=ALU.mult)
        sqn = fin_pool.tile([128, KF], dt.float32, tag="sqn")
        nc.vector.tensor_tensor(out=sqn[:], in0=vn[:], in1=vn[:], op=ALU.mult)

        gp = finp_pool.tile([128, KF], dt.float32, tag="red")
        nc.tensor.matmul(gp[:], lhsT=onesf[:], rhs=sqn[:],
                         start=True, stop=True)
        g2 = fin_pool.tile([128, IMGS], dt.float32, tag="g2")
        nc.vector.tensor_reduce(
            out=g2[:], in_=gp[:].rearrange("p (n k) -> p n k", k=K),
            axis=mybir.AxisListType.X, op=ALU.add,
        )
        lg2 = fin_pool.tile([128, IMGS], dt.float32, tag="lg2")
        nc.scalar.activation(lg2[:], g2[:], ACTF.Ln)
        ginv = fin_pool.tile([128, IMGS], dt.float32, tag="ginv")
        nc.scalar.activation(ginv[:], lg2[:], ACTF.Exp, scale=-0.5)

        fin = fin_pool.tile([128, KF], dt.float32, tag="fin")
        nc.vector.tensor_tensor(
            out=fin[:].rearrange("p (n k) -> p n k", k=K),
            in0=vn[:].rearrange("p (n k) -> p n k", k=K),
            in1=ginv[:].unsqueeze(2).broadcast_to((128, IMGS, K)),
            op=ALU.mult,
        )

        # transpose [128d, 32k] -> [32k, 128d] per image, pack 4 per bank
        for b in range(2):
            otp = ot_pool.tile([32, 512], dt.float32, tag="ot")
            for j in range(4):
                i = 4 * b + j
                nc.tensor.matmul(
                    otp[:, 128 * j:128 * j + 128],
                    lhsT=fin[:, K * i:K * i + K],
                    rhs=identf[:],
                    is_transpose=True,
                    start=(j == 0), stop=(j == 3),
                )
            osb = fin_pool.tile([32, 512], dt.float32, tag="osb")
            nc.scalar.copy(out=osb[:], in_=otp[:])
            nc.sync.dma_start(
                out=out_d[4 * b:4 * b + 4].rearrange("n (k d) -> k n d", k=K),
                in_=osb[:].rearrange("k (n d) -> k n d", n=4),
            )

    es.close()


def build_nc(debug: bool = False):
    _patch_act_tables()
    nc = bacc.Bacc(
        "TRN2",
        target_bir_lowering=False,
        debug=debug,
        enable_asserts=debug,
        num_devices=N_CORES,
    )
    io = {
        "x": nc.dram_tensor("x", [IMGS, D, HW], dt.bfloat16,
                            kind="ExternalInput").ap(),
        "wT": nc.dram_tensor("wT", [128, K], dt.bfloat16,
                             kind="ExternalInput").ap(),
        "identf": nc.dram_tensor("identf", [128, 128], dt.float32,
                                 kind="ExternalInput").ap(),
        "eb": nc.dram_tensor("eb", [IMGS, IMGS * 128], dt.float32,
                             kind="ExternalInput").ap(),
        "onesf": nc.dram_tensor("onesf", [128, 128], dt.float32,
                                kind="ExternalInput").ap(),
        "cT8": nc.dram_tensor("cT8", [128, IMGS * K], dt.float32,
                              kind="ExternalInput").ap(),
        "out": nc.dram_tensor("out", [IMGS, HW], dt.float32,
                              kind="ExternalOutput").ap(),
    }
    with tile.TileContext(nc) as tc:
        _emit(tc, io)
    nc.finalize()
    return nc


def make_in_maps(x, conv_w, centroids):
    """Host-side shard + param prep. x [64,128,64,64] f32."""
    bf = ml_dtypes.bfloat16
    xf = np.ascontiguousarray(x.reshape(N, D, HW)).astype(np.float32).astype(bf)
    consts = {
        "wT": np.ascontiguousarray(conv_w.T).astype(bf),
        "identf": np.eye(128, dtype=np.float32),
        "eb": np.kron(np.eye(IMGS, dtype=np.float32),
                      np.ones((1, 128), np.float32)),
        "onesf": np.ones((128, 128), np.float32),
        "cT8": np.ascontiguousarray(
            np.tile(centroids.T.astype(np.float32), (1, IMGS))),
    }
    in_maps = []
    for core in range(N_CORES):
        m = dict(consts)
        m["x"] = np.ascontiguousarray(xf[IMGS * core:IMGS * (core + 1)])
        in_maps.append(m)
    return in_maps


_NC_CACHE = None


def kernel(x, conv_w, centroids):
    global _NC_CACHE
    if _NC_CACHE is None:
        _NC_CACHE = build_nc()
    nc = _NC_CACHE
    in_maps = make_in_maps(np.asarray(x), np.asarray(conv_w),
                           np.asarray(centroids))
    res = run_bass_kernel_spmd(nc, in_maps, list(range(N_CORES)))
    out = np.concatenate([np.asarray(res.results[i]["out"])
                          for i in range(N_CORES)], axis=0)
    return out.reshape(N, K * D).astype(np.float32)
